# revision 1
# baseline (speedup 1.0000x reference)
"""Trainium2 Bass kernel for nn_Concat4 (topk channel sort + gather + tail fold).

reference semantics (per sample):
    x = concat([x1, x2], channel axis)            # [512, 64*64]
    pooled = mean(x, spatial)                     # [512]
    order = argsort(-pooled, stable)              # descending, stable
    xr = x[order]                                 # sorted channels
    out = xr[:k];  out[k-1] += xr[k:].sum(0)

Strategy (pure data parallel, 2 samples per core on 8 cores):
  - load each sample's 512 channels into ONE SBUF tile [128, 4*4096]
    (channel c = g*128 + p lives at partition p, columns [g*4096,(g+1)*4096))
  - accurate 3-stage f32 row sums -> pooled[128, 4]  (order must match the
    reference's f32 argsort; multi-stage keeps noise ~1e-5 << min gap)
  - rank every channel with O(C^2) comparisons on the vector engine
    (count of strictly-greater pooled + stable tie-break on channel index)
  - channels with rank <= k-2: ONE indirect-DMA scatter per sample
    SBUF row-part -> out[b*k + rank]  (rank > k-2 skipped via bounds check)
  - channels with rank >= k-1: fp32 matmul with a 0/1 mask column sums
    them into one tail row == xr[k-1] + xr[k:].sum(0), DMA'd to out[k-1]

Uses bacc.Bacc + nc.compile() so multi-sem sync waits are legalized into
event-semaphore instructions (plain bass.Bass trips walrus's one-wait-per-
instruction limit).
"""

import numpy as np

HW = 4096  # 64*64
P = 128
C_IN = 256
C_TOT = 512
NG = C_TOT // P  # 4 channel groups per sample
W_TOT = NG * HW  # 16384 columns in the per-sample SBUF tile
B = 16
N_CORES = 8
B_LOC = B // N_CORES  # samples per core

_CACHE = {}


def _build(k: int, reps: int = 1):
    import concourse.bass as bass
    import concourse.bacc as bacc
    import concourse.mybir as mybir
    from concourse.bass import IndirectOffsetOnAxis
    from concourse.tile import TileContext

    f32 = mybir.dt.float32
    i32 = mybir.dt.int32
    bf16 = mybir.dt.bfloat16
    AX = mybir.AxisListType.X
    OP = mybir.AluOpType

    nc = bacc.Bacc()
    x1 = nc.declare_dram_parameter("x1", [B_LOC, C_IN, HW], f32, isOutput=False)
    x2 = nc.declare_dram_parameter("x2", [B_LOC, C_IN, HW], f32, isOutput=False)
    out = nc.declare_dram_parameter("out", [B_LOC * k, HW], f32, isOutput=True)

    with TileContext(nc) as tc:
        with (
            tc.tile_pool(name="xp", bufs=2) as xp,
            tc.tile_pool(name="wide", bufs=3) as wide,
            tc.tile_pool(name="small", bufs=4) as small,
            tc.tile_pool(name="consts", bufs=1) as consts,
            tc.tile_pool(name="tailp", bufs=2) as tailp,
            tc.tile_pool(name="dramp", bufs=2, space="DRAM") as dramp,
            tc.tile_pool(name="psum", bufs=1, space="PSUM") as psump,
        ):
            # ---- constants ----------------------------------------------
            # free position j' = (p', g') holds channel c' = g'*128 + p'
            chan_i = consts.tile([P, C_TOT], i32, tag="chan_i")
            nc.gpsimd.iota(chan_i[:], pattern=[[1, P], [P, NG]], base=0,
                           channel_multiplier=0)
            chan_f = consts.tile([P, C_TOT], f32, tag="chan_f")
            nc.vector.tensor_copy(chan_f[:], chan_i[:])
            # per-partition channel id for group g: c = g*128 + p
            cid_i = consts.tile([P, NG], i32, tag="cid_i")
            nc.gpsimd.iota(cid_i[:], pattern=[[P, NG]], base=0,
                           channel_multiplier=1)
            cid_f = consts.tile([P, NG], f32, tag="cid_f")
            nc.vector.tensor_copy(cid_f[:], cid_i[:])
            # lt_g[p, j'] = 1.0 where c'(j') < c(p, g)   (constant masks)
            lt = []
            for g in range(NG):
                lt_g = consts.tile([P, C_TOT], f32, tag=f"lt{g}")
                nc.vector.tensor_scalar(lt_g[:], chan_f[:], cid_f[:, g:g + 1],
                                        None, op0=OP.is_lt)
                lt.append(lt_g)

            tail_row_prev = None
            for rep in range(reps):
              loaded = []
              for b in range(B_LOC):
                # ---- load sample into one [128, 16384] tile --------------
                # (all loads + stage-1 reduces traced before any per-sample
                # processing so the DMA queues never starve, and each half's
                # reduce overlaps the other half's load)
                bigx = xp.tile([P, W_TOT], f32, tag="bigx")
                s1 = wide.tile([P, NG * 256], f32, tag="s1")
                for h, src_t in enumerate((x1, x2)):
                    nc.sync.dma_start(
                        out=bigx[:, h * 2 * HW:(h + 1) * 2 * HW]
                            .rearrange("p (gg w) -> p gg w", gg=2),
                        in_=src_t[b].rearrange("(gg p) w -> p gg w", p=P))
                    nc.vector.reduce_sum(
                        s1[:, h * 512:(h + 1) * 512]
                            .rearrange("p (g a) -> p g a", g=2),
                        bigx[:, h * 2 * HW:(h + 1) * 2 * HW]
                            .rearrange("p (g a c) -> p g a c", g=2, a=256),
                        axis=AX)
                loaded.append((bigx, s1))
              for b in range(B_LOC):
                bigx, s1 = loaded[b]
                # ---- pooled sums: finish 3-stage accurate reduce ---------
                s2 = small.tile([P, NG * 16], f32, tag="s2")
                pooled = small.tile([P, NG], f32, tag="pooled")
                nc.vector.reduce_sum(
                    s2[:].rearrange("p (g a) -> p g a", g=NG),
                    s1[:].rearrange("p (g a c) -> p g a c", g=NG, a=16),
                    axis=AX)
                nc.vector.reduce_sum(
                    pooled[:, :, None],
                    s2[:].rearrange("p (g c) -> p g c", g=NG),
                    axis=AX)

                # ---- broadcast pooled to all partitions via DRAM bounce --
                pscr = dramp.tile([P, NG], f32, tag="pscr")
                nc.gpsimd.dma_start(out=pscr[:], in_=pooled[:])
                bc = wide.tile([P, C_TOT], f32, tag="bc")
                nc.gpsimd.dma_start(
                    out=bc[:],
                    in_=pscr[:].rearrange("p g -> (p g)")[None, :]
                        .to_broadcast((P, C_TOT)))

                # ---- ranks, masks, scatter indices ----------------------
                masks = small.tile([P, NG], f32, tag="masks")
                idx = small.tile([P, NG], i32, tag="idx")
                for g in range(NG):
                    scr1 = wide.tile([P, C_TOT], f32, tag="scr")
                    scr2 = wide.tile([P, C_TOT], f32, tag="scr")
                    rgt = small.tile([P, 1], f32, tag="rgt")
                    rtie = small.tile([P, 1], f32, tag="rtie")
                    # rank_gt[p] = #{c' : pooled[c'] > pooled[c]}
                    nc.vector.tensor_scalar(scr1[:], bc[:], pooled[:, g:g + 1],
                                            None, op0=OP.is_gt, op1=OP.add,
                                            accum_out=rgt[:])
                    # rank_tie[p] = #{c' < c : pooled[c'] == pooled[c]}
                    nc.vector.scalar_tensor_tensor(
                        scr2[:], bc[:], pooled[:, g:g + 1], lt[g][:],
                        op0=OP.is_equal, op1=OP.mult, accum_out=rtie[:])
                    rank = small.tile([P, 1], f32, tag="rank")
                    nc.vector.tensor_add(rank[:], rgt[:], rtie[:])
                    nc.vector.tensor_scalar(masks[:, g:g + 1], rank[:],
                                            float(k - 1), None, op0=OP.is_ge)
                    idx_f = small.tile([P, 1], f32, tag="idxf")
                    nc.vector.tensor_scalar(idx_f[:], rank[:], float(b * k),
                                            None, op0=OP.add)
                    nc.vector.tensor_copy(idx[:, g:g + 1], idx_f[:])

                # ---- scatter channels with rank <= k-2 ------------------
                # NOTE: the HW indirect DMA takes ONE index per partition and
                # moves that partition's whole free extent, so one scatter per
                # channel group.
                if k >= 2:
                    for g in range(NG):
                        nc.gpsimd.indirect_dma_start(
                            out=out[:, :],
                            out_offset=IndirectOffsetOnAxis(ap=idx[:, g:g + 1],
                                                            axis=0),
                            in_=bigx[:, g * HW:(g + 1) * HW],
                            in_offset=None,
                            bounds_check=b * k + (k - 2),
                            oob_is_err=False,
                        )

                # ---- tail row: sum of channels with rank >= k-1 ---------
                tail_row = tailp.tile([1, HW], f32, tag="tailrow")
                pts = []
                for n in range(8):
                    pt = psump.tile([1, 512], f32, tag=f"pt{n}")
                    pts.append(pt)
                for g in range(NG):
                    for n in range(8):
                        nc.tensor.matmul(
                            pts[n][0:1, :],
                            lhsT=masks[:, g:g + 1],
                            rhs=bigx[:, g * HW + n * 512:g * HW + (n + 1) * 512],
                            start=(g == 0),
                            stop=(g == NG - 1),
                        )
                for n in range(8):
                    nc.scalar.copy(tail_row[0:1, n * 512:(n + 1) * 512],
                                   pts[n][0:1, :])
                row = b * k + (k - 1)
                nc.gpsimd.dma_start(out=out[row:row + 1, :], in_=tail_row[0:1, :])
                tail_row_prev = tail_row

    nc.compile()
    return nc


def _get(k: int):
    if k not in _CACHE:
        _CACHE[k] = _build(k)
    return _CACHE[k]


def kernel(x1, x2, k):
    from concourse.bass_utils import run_bass_kernel_spmd

    k = int(k)
    x1 = np.ascontiguousarray(np.asarray(x1), dtype=np.float32)
    x2 = np.ascontiguousarray(np.asarray(x2), dtype=np.float32)
    assert x1.shape == (B, C_IN, 64, 64) and x2.shape == (B, C_IN, 64, 64)

    x1f = x1.reshape(B, C_IN, HW)
    x2f = x2.reshape(B, C_IN, HW)
    nc = _get(k)
    in_maps = [
        {"x1": x1f[i * B_LOC:(i + 1) * B_LOC], "x2": x2f[i * B_LOC:(i + 1) * B_LOC]}
        for i in range(N_CORES)
    ]
    res = run_bass_kernel_spmd(nc, in_maps, list(range(N_CORES)))
    parts = [res.results[i]["out"].reshape(B_LOC, k, 64, 64)
             for i in range(N_CORES)]
    return np.concatenate(parts, axis=0)

